# revision 1
# baseline (speedup 1.0000x reference)
"""Trainium2 Bass kernel for the branched cross-attention processor.

Problem (full shapes):
  hidden_states [4, 4096, 1280], encoder_hidden_states [4, 77, 2048],
  id_embedding [2, 32, 2048], Wq/Wout [1280,1280], Wk/Wv/Wid_k/Wid_v
  [2048,1280], bout [1280].  20 heads, dh=64.  Output [4, 4096, 1280].

Sharding: data-parallel over (batch, seq-half): core c handles batch c//2,
query rows (c%2)*2048 : (c%2+1)*2048.  K/V (109 keys) are computed
per-core for its batch.  All queries are independent (full cross
attention), so no collectives are needed.

Per-core pipeline (all matmuls float32r, N=512 => full PE rate):
  qT[j,sq]    = sum_i Wq[i,j] * hsT[i,sq]                (q projection, transposed)
  k~/v~       = [ehs @ Wk|Wv ; id @ Wid_k|Wid_v]          (109 keys, padded to 128
                rows: [0:77]=ehs, [77:96]=zero gap, [96:128]=id)
  kT          = transpose(k~)  (PE transpose)
  scoresT     = kT_h^T @ qT_h        [128keys, 512sq]  per (head, sq-chunk)
  probsT      = exp(0.125*scoresT + gapbias)   (gap rows get -1e30 -> exp 0)
  attnT_pair  = v_pair^T @ probsT    [128, 512]  (head h in rows 64*(h%2):+64)
  denom_rep   = ones128^T @ probsT   [128, 512]  (denominator replicated on all
                partitions by an all-ones stationary matrix -> no broadcast op)
  attnT_h     = attnT_pair[rows] * reciprocal(denom_rep)[rows]
  out         = attnT^T @ Wout + bout

fp32r ISA restriction: matmul dst must cover all 128 partitions (col_grp
0xf), so every matmul here has M=128; partial results use only the rows
that are valid.
"""

import os
import sys
import types

import numpy as np

# ---------------------------------------------------------------------------
# problem constants (hardcoded; kernel.py must be self-contained)
# ---------------------------------------------------------------------------
B = 4
S = 4096
H = 1280
C = 2048
TE = 77          # encoder tokens
TI = 32          # id tokens
HEADS = 20
DH = 64          # head dim
P = 128
L = 109          # TE + TI
LP = 128         # padded key count: [0:77]=ehs, [77:96]=gap, [96:128]=id
GAP0, GAP1 = TE, P - TI   # 77, 96
SC = 2048        # seq rows per core
NJ = H // P      # 10
NI = C // P      # 16
NCH = SC // 512  # 4 sq-chunks of 512
NT = SC // P     # 16 sq-tiles of 128
SCALE = 1.0 / 8.0
NCORES = 8
MCHUNKS = [(0, 512), (512, 512), (1024, 256)]

_NC_CACHE = {}


def _ensure_axon_hooks():
    """The image's antenv lacks axon_hooks; synthesize it so NTFF profiling
    (trace=True) works when test.py asks for it.  Harmless if unused."""
    if "antenv.axon_hooks" in sys.modules:
        return
    try:
        import antenv
        from trn_agent_boot.trn_boot import _ntff_profile_via_ctypes

        hook = _ntff_profile_via_ctypes("/opt/axon/libaxon_pjrt.so")
        m = types.ModuleType("antenv.axon_hooks")
        m.get_axon_ntff_profile_hook = lambda: hook
        m.set_axon_ntff_profile_hook = lambda h: None
        sys.modules["antenv.axon_hooks"] = m
        antenv.axon_hooks = m
    except Exception:
        pass


def build_nc():
    """Build + compile the per-core Bass program (SPMD: same NEFF, 8 cores)."""
    if "nc" in _NC_CACHE:
        return _NC_CACHE["nc"]

    import concourse.bass as bass
    import concourse.tile as tile
    from concourse import bacc, mybir
    from concourse.bass import ts

    F32 = mybir.dt.float32
    R = mybir.dt.float16      # matmul operand dtype (1 cyc/row, 10-bit mantissa)
    EXP = mybir.ActivationFunctionType.Exp

    nc = bacc.Bacc("TRN2", target_bir_lowering=False, debug=False, num_devices=NCORES)

    hsT = nc.dram_tensor("hsT", [H, SC], R, kind="ExternalInput").ap()
    xkvTp = nc.dram_tensor("xkvTp", [NI, P, LP], R, kind="ExternalInput").ap()
    wqp = nc.dram_tensor("wqp", [NJ, NJ, P, P], R, kind="ExternalInput").ap()
    wkvp = nc.dram_tensor("wkvp", [5, NI, P, 512], R, kind="ExternalInput").ap()
    widkvp = nc.dram_tensor("widkvp", [5, NI, P, 512], R, kind="ExternalInput").ap()
    woutT = nc.dram_tensor("woutT", [H, H], R, kind="ExternalInput").ap()
    boutb = nc.dram_tensor("boutb", [P, H], F32, kind="ExternalInput").ap()
    out = nc.dram_tensor("out", [SC, H], F32, kind="ExternalOutput").ap()

    with tile.TileContext(nc) as tc:
        with tc.tile_pool(name="pers", bufs=1) as pers:
            qTp_cm = tc.tile_pool(name="qTp", bufs=1)
            qTp = qTp_cm.__enter__()
            # ---- persistent constants / arrays --------------------------------
            ones_mat = pers.tile([P, P], R, tag="ones_mat")
            nc.vector.memset(ones_mat[:, :], 1.0)
            bias_col = pers.tile([P, 1], F32, tag="bias_col")
            # engine ops need 32-aligned start partitions: write the gap
            # as [64:96] then restore [64:77]; later writes overwrite cleanly.
            nc.vector.memset(bias_col[:, :], 0.0)
            nc.vector.memset(bias_col[64:GAP1, :], -1e30)
            nc.vector.memset(bias_col[64:GAP0, :], 0.0)
            kT_sb = [pers.tile([P, LP], R, tag=f"kT{j}", name=f"kT{j}") for j in range(NJ)]
            v_sb = pers.tile([LP, HEADS * DH], R, tag="v")

            qT_sb = [qTp.tile([P, SC], R, tag=f"qT{j}", name=f"qT{j}") for j in range(NJ)]

            # ---- phase Q: q projection + k/v half-projection + pair exchange --
            with (
                tc.tile_pool(name="phq", bufs=1) as phq,
                tc.tile_pool(name="wqs", bufs=6) as wqs,
                tc.tile_pool(name="wkvs", bufs=4) as wkvs,
                tc.tile_pool(name="psq", bufs=4, space="PSUM") as psq,
                tc.tile_pool(name="pskv", bufs=2, space="PSUM") as pskv,
            ):
                hsT_sb = [phq.tile([P, SC], R, tag=f"hsT{i}", name=f"hsT{i}") for i in range(NJ)]
                for i in range(NJ):
                    nc.sync.dma_start(out=hsT_sb[i][:, :], in_=hsT[ts(i, P), :])
                xkvT_sb = [phq.tile([P, LP], R, tag=f"xkvT{i}", name=f"xkvT{i}") for i in range(NI)]
                for i in range(NI):
                    nc.sync.dma_start(out=xkvT_sb[i][:, :], in_=xkvTp[i])
                kTMP = phq.tile([P, H], R, tag="kTMP")

                def q_group(j):
                    pss = [psq.tile([P, 512], F32, tag="qps", name="qps") for _ in range(NCH)]
                    for i in range(NJ):
                        wq_t = wqs.tile([P, P], R, tag="wq", name="wq_t")
                        nc.sync.dma_start(out=wq_t[:, :], in_=wqp[j, i])
                        for c in range(NCH):
                            nc.tensor.matmul(
                                pss[c][:, :], wq_t[:, :], hsT_sb[i][:, ts(c, 512)],
                                start=(i == 0), stop=(i == NJ - 1),
                            )
                    for c in range(NCH):
                        nc.scalar.copy(qT_sb[j][:, ts(c, 512)], pss[c][:, :])

                # kv projection, chunk (proj, n) at a time; chunks ordered
                # k-columns-first so kT transposes start while v still streams.
                # proj 0 = [Wk|Wv] (valid rows 0:77 + zero gap), proj 1 =
                # [Wid_k|Wid_v] (valid rows 96:128, overwrites after proj 0).
                def kv_chunk(proj, n):
                    srcw = wkvp if proj == 0 else widkvp
                    ps = pskv.tile([P, 512], F32, tag="kvps", name="kvps")
                    for i in range(NI):
                        w_t = wkvs.tile([P, 512], R, tag="wkv", name="wkv_t")
                        nc.sync.dma_start(out=w_t[:, :], in_=srcw[n, i])
                        nc.tensor.matmul(
                            ps[:, :], xkvT_sb[i][:, :], w_t[:, :],
                            start=(i == 0), stop=(i == NI - 1),
                        )
                    lo, hi = (0, P) if proj == 0 else (GAP1, P)
                    if n < 2:
                        nc.scalar.copy(kTMP[lo:hi, ts(n, 512)], ps[lo:hi, :])
                    elif n == 2:
                        nc.scalar.copy(kTMP[lo:hi, 1024:1280], ps[lo:hi, 0:256])
                        nc.scalar.copy(v_sb[lo:hi, 0:256], ps[lo:hi, 256:512])
                    else:
                        v0 = 512 * n - 1280
                        nc.scalar.copy(v_sb[lo:hi, v0:v0 + 512], ps[lo:hi, :])
                    if (proj, n) == (1, 2):
                        # all k columns final -> transpose k~ into kT
                        for j in range(NJ):
                            nc.sync.dma_start(out=kT_sb[j][:, :],
                                              in_=kTMP[:, ts(j, P)], transpose=True)

                # k chunks first (both projs), then v chunks
                kv_plan = [(0, 0), (0, 1), (0, 2), (1, 0), (1, 1), (1, 2),
                           (0, 3), (0, 4), (1, 3), (1, 4)]
                kv_at = {0: [0], 1: [1], 2: [2], 3: [3], 4: [4], 5: [5],
                         6: [6], 7: [7], 8: [8], 9: [9]}
                for j in range(NJ):
                    q_group(j)
                    for kvi in kv_at.get(j, []):
                        kv_chunk(*kv_plan[kvi])

            # ---- phase A: attention -------------------------------------------
            attnp_cm = tc.tile_pool(name="attnp", bufs=1, side="right")
            attnp = attnp_cm.__enter__()
            attnT_sb = [attnp.tile([P, SC], R, tag=f"attnT{d}", name=f"attnT{d}") for d in range(NJ)]
            boutb_sb = attnp.tile([P, H], F32, tag="boutb")
            nc.sync.dma_start(out=boutb_sb[:, :], in_=boutb)
            wout_sb = [attnp.tile([P, H], R, tag=f"wout{i}", name=f"wout{i}") for i in range(NJ)]
            for i in range(NJ):
                nc.sync.dma_start(out=wout_sb[i][:, :], in_=woutT[ts(i, P), :])
            with (
                tc.tile_pool(name="pha", bufs=3) as pha,
                tc.tile_pool(name="psa", bufs=2, space="PSUM") as psa,
            ):
                # software pipeline over head-pairs: scores+exp of pair p
                # run while PV/denominator/normalize of pair p-1 occupy the
                # PE/DVE, so the PE never waits on the ACT exp latency.  The
                # two scores matmuls of a pair sit in different PE row groups
                # (rows 0:64 / 64:128) and can overlap in the array.
                pairs = [(c, hp) for c in range(NCH) for hp in range(NJ)]
                astate = {}

                def attn_front(idx):
                    c, hp = pairs[idx]
                    pts = []
                    for s in range(2):
                        rq = DH * s
                        ps_s = psa.tile([P, 512], F32, tag="sps", name="sps")
                        nc.tensor.matmul(
                            ps_s[:, :], kT_sb[hp][rq:rq + DH, :],
                            qT_sb[hp][rq:rq + DH, ts(c, 512)],
                            start=True, stop=True,
                        )
                        pts.append(ps_s)
                    probs = []
                    for s in range(2):
                        probsT = pha.tile([P, 512], R, tag="probsT", name="probsT")
                        nc.scalar.activation(
                            probsT[:, :], pts[s][:, :], EXP,
                            bias=bias_col[:, :], scale=SCALE,
                        )
                        probs.append(probsT)
                    astate[idx] = probs

                def attn_back(idx):
                    c, hp = pairs[idx]
                    probs = astate.pop(idx)
                    # both heads share one PV psum tile (disjoint row halves)
                    # and one denominator tile (denom_h replicated over its
                    # own half by a ones stationary).
                    ps_o = psa.tile([P, 512], F32, tag="ops", name="ops")
                    ps_d = psa.tile([P, 512], F32, tag="dps", name="dps")
                    for s in range(2):
                        h = 2 * hp + s
                        rq = DH * s
                        nc.tensor.matmul(
                            ps_o[rq:rq + DH, :], v_sb[:, ts(h, DH)], probs[s][:, :],
                            start=True, stop=True,
                        )
                        nc.tensor.matmul(
                            ps_d[rq:rq + DH, :], ones_mat[:, 0:DH], probs[s][:, :],
                            start=True, stop=True,
                        )
                    bc_sb = pha.tile([P, 512], F32, tag="bc", name="bc_sb")
                    nc.vector.reciprocal_approx_fast(bc_sb[:, :], ps_d[:, :])
                    nc.vector.tensor_mul(
                        attnT_sb[hp][:, ts(c, 512)], ps_o[:, :], bc_sb[:, :]
                    )

                for idx in range(len(pairs)):
                    attn_front(idx)
                    if idx >= 1:
                        attn_back(idx - 1)
                attn_back(len(pairs) - 1)

            # release qT before phase O (attnp on the right stack stays open)
            qTp_cm.__exit__(None, None, None)

            # ---- phase O: output projection + bias ----------------------------
            with (
                tc.tile_pool(name="finp", bufs=3) as finp,
                tc.tile_pool(name="pso", bufs=4, space="PSUM") as pso,
            ):
                for t in range(NT):
                    fin = finp.tile([P, H], F32, tag="fin", name="fin")
                    for m0, mw in MCHUNKS:
                        psf = pso.tile([P, mw], F32, tag="psf", name="psf")
                        for i in range(NJ):
                            nc.tensor.matmul(
                                psf[:, :], attnT_sb[i][:, ts(t, P)],
                                wout_sb[i][:, m0:m0 + mw],
                                start=(i == 0), stop=(i == NJ - 1),
                            )
                        nc.vector.tensor_add(
                            fin[:, m0:m0 + mw], psf[:, :], boutb_sb[:, m0:m0 + mw]
                        )
                    nc.sync.dma_start(out=out[ts(t, P), :], in_=fin[:, :])

            attnp_cm.__exit__(None, None, None)

    nc.compile()
    _NC_CACHE["nc"] = nc
    return nc


def prep_core_inputs(hidden_states, encoder_hidden_states, id_embedding,
                     Wq, Wk, Wv, Wid_k, Wid_v, Wout, bout):
    """Host-side sharding / layout prep.  Returns list of 8 in_maps."""
    f = np.float32
    h16 = np.float16
    hidden_states = np.asarray(hidden_states, f)
    encoder_hidden_states = np.asarray(encoder_hidden_states, f)
    id_embedding = np.asarray(id_embedding, f)
    Wq = np.asarray(Wq, f)
    Wout = np.asarray(Wout, f)
    Wk, Wv = np.asarray(Wk, f), np.asarray(Wv, f)
    Wid_k, Wid_v = np.asarray(Wid_k, f), np.asarray(Wid_v, f)
    boutb = np.ascontiguousarray(np.broadcast_to(np.asarray(bout, f), (P, H)))

    # packed tile-major weight layouts (contiguous DMA tiles)
    wqp = np.ascontiguousarray(
        Wq.reshape(NJ, P, NJ, P).transpose(2, 0, 1, 3).astype(h16))            # [j,i,128,128]
    wkv = np.concatenate([Wk, Wv], axis=1)                                     # [C, 2H]
    widkv = np.concatenate([Wid_k, Wid_v], axis=1)
    wkvp = np.ascontiguousarray(
        wkv.reshape(NI, P, 5, 512).transpose(2, 0, 1, 3).astype(h16))          # [n,i,128,512]
    widkvp = np.ascontiguousarray(
        widkv.reshape(NI, P, 5, 512).transpose(2, 0, 1, 3).astype(h16))

    wout16 = np.ascontiguousarray(Wout.astype(h16))
    in_maps = []
    for core in range(NCORES):
        b, hf = divmod(core, 2)
        hsT = np.ascontiguousarray(hidden_states[b, hf * SC:(hf + 1) * SC, :].T.astype(h16))
        xkvT = np.zeros((C, LP), h16)                                          # [C, 128]
        xkvT[:, :TE] = encoder_hidden_states[b].T
        xkvT[:, GAP1:] = id_embedding[b % 2].T
        xkvTp = np.ascontiguousarray(xkvT.reshape(NI, P, LP))
        in_maps.append({
            "hsT": hsT, "xkvTp": xkvTp, "wqp": wqp, "wkvp": wkvp,
            "widkvp": widkvp, "woutT": wout16, "boutb": boutb,
        })
    return in_maps


def kernel(hidden_states, encoder_hidden_states, id_embedding,
           Wq, Wk, Wv, Wid_k, Wid_v, Wout, bout, _trace=False):
    _ensure_axon_hooks()
    from concourse.bass_utils import run_bass_kernel_spmd

    nc = build_nc()
    in_maps = prep_core_inputs(hidden_states, encoder_hidden_states, id_embedding,
                               Wq, Wk, Wv, Wid_k, Wid_v, Wout, bout)
    kwargs = {}
    if _trace:
        import concourse.bass_utils as bu
        bu.upload_artifacts = lambda tmpdir: f"local://{tmpdir}"
        kwargs["trace"] = True
    res = run_bass_kernel_spmd(nc, in_maps, core_ids=list(range(NCORES)), **kwargs)

    outp = np.empty((B, S, H), np.float32)
    for core in range(NCORES):
        b, hf = divmod(core, 2)
        outp[b, hf * SC:(hf + 1) * SC, :] = res.results[core]["out"]
    if _trace:
        kernel.last_exec_time_ns = res.exec_time_ns
        kernel.last_results = res
    return outp



# revision 3
# speedup vs baseline: 1.5191x; 1.5191x over previous
"""Trainium2 Bass kernel for the branched cross-attention processor.

Problem (full shapes):
  hidden_states [4, 4096, 1280], encoder_hidden_states [4, 77, 2048],
  id_embedding [2, 32, 2048], Wq/Wout [1280,1280], Wk/Wv/Wid_k/Wid_v
  [2048,1280], bout [1280].  20 heads, dh=64.  Output [4, 4096, 1280].

Sharding: data-parallel over (batch, seq-half): core c handles batch c//2,
query rows (c%2)*2048 : (c%2+1)*2048.  K/V (109 keys padded to 128:
[0:77]=ehs, [77:96]=zero gap, [96:128]=id) are computed per-core for its
batch.  No collectives.

Schedule: a 3-deep software pipeline over 4 query chunks of 512 keeps the
PE dense (TRN2 drops the PE clock from 2.4 to 1.2 GHz for ~3us after any
stall, so every bubble costs ~1.5us).  Chunk-slot t runs, interleaved at
head-pair granularity:
    Q-projection of chunk t | attention of chunk t-1 | out-proj of t-2
The kv projection (10 weight chunks) fills chunk-slot 0.  Attention
per pair: scoresT = kT^T qT -> exp (ACT, gap-masked bias) -> PV + ones
matmul denominator (PE) -> reciprocal+normalize (DVE).  The exp/recip/mul
engine work hides under the Q/O matmuls of the same slot.

DMAs are batched into ~60 large transfers (the per-dma_start trigger is
~0.6us of SP sequencer time) and issued in arrival-priority order.
"""

import os
import sys
import types

import numpy as np

# ---------------------------------------------------------------------------
# problem constants (hardcoded; kernel.py must be self-contained)
# ---------------------------------------------------------------------------
B = 4
S = 4096
H = 1280
C = 2048
TE = 77          # encoder tokens
TI = 32          # id tokens
HEADS = 20
DH = 64          # head dim
P = 128
L = 109          # TE + TI
LP = 128         # padded key count
GAP0, GAP1 = TE, P - TI   # 77, 96
SC = 2048        # seq rows per core
NJ = H // P      # 10
NI = C // P      # 16
NCH = SC // 512  # 4 query chunks of 512
NT = SC // P     # 16 q-tiles of 128
SCALE = 1.0 / 8.0
NCORES = 8
NPAIR = NCH * NJ  # 40 (chunk, head-pair) attention units
# kv chunk plan: proj 0 = [Wk|Wv] (ehs rows), proj 1 = [Wid_k|Wid_v]
# (id rows).  k columns first so kT transposes can fire at index 5.
KV_PLAN = [(0, 0), (0, 1), (0, 2), (1, 0), (1, 1), (1, 2),
           (0, 3), (0, 4), (1, 3), (1, 4)]

_NC_CACHE = {}


def _ensure_axon_hooks():
    """The image's antenv lacks axon_hooks; synthesize it so NTFF profiling
    (trace=True) works when test.py asks for it.  Harmless if unused."""
    if "antenv.axon_hooks" in sys.modules:
        return
    try:
        import antenv
        from trn_agent_boot.trn_boot import _ntff_profile_via_ctypes

        hook = _ntff_profile_via_ctypes("/opt/axon/libaxon_pjrt.so")
        m = types.ModuleType("antenv.axon_hooks")
        m.get_axon_ntff_profile_hook = lambda: hook
        m.set_axon_ntff_profile_hook = lambda h: None
        sys.modules["antenv.axon_hooks"] = m
        antenv.axon_hooks = m
    except Exception:
        pass


def build_nc():
    """Build + compile the per-core Bass program (SPMD: same NEFF, 8 cores)."""
    if "nc" in _NC_CACHE:
        return _NC_CACHE["nc"]

    import concourse.bass as bass
    import concourse.tile as tile
    from concourse import bacc, mybir
    from concourse.bass import ts

    F32 = mybir.dt.float32
    R = mybir.dt.float16      # matmul operand dtype (1 cyc/row)
    EXP = mybir.ActivationFunctionType.Exp

    nc = bacc.Bacc("TRN2", target_bir_lowering=False, debug=False, num_devices=NCORES)

    hsTp = nc.dram_tensor("hsTp", [NCH, P, NJ * 512], R, kind="ExternalInput").ap()
    xkvp = nc.dram_tensor("xkvp", [P, NI * LP], R, kind="ExternalInput").ap()
    wqp = nc.dram_tensor("wqp", [NJ, P, NJ * P], R, kind="ExternalInput").ap()
    wkvh = nc.dram_tensor("wkvh", [10, 2, P, 8 * 512], R, kind="ExternalInput").ap()
    woutp = nc.dram_tensor("woutp", [P, NJ * H], R, kind="ExternalInput").ap()
    boutb = nc.dram_tensor("boutb", [P, H], F32, kind="ExternalInput").ap()
    out = nc.dram_tensor("out", [SC, H], F32, kind="ExternalOutput").ap()

    with tile.TileContext(nc) as tc:
        with (
            tc.tile_pool(name="pers", bufs=1) as pers,
            tc.tile_pool(name="hsp", bufs=2) as hsp,
            tc.tile_pool(name="qtp", bufs=2) as qtp,
            tc.tile_pool(name="atp", bufs=2) as atp,
            tc.tile_pool(name="kvwp", bufs=4) as kvwp,
            tc.tile_pool(name="prp", bufs=6) as prp,
            tc.tile_pool(name="bcp", bufs=2) as bcp,
            tc.tile_pool(name="finp", bufs=3) as finp,
            tc.tile_pool(name="psA", bufs=3, space="PSUM") as psA,
            tc.tile_pool(name="psS", bufs=3, space="PSUM") as psS,
            tc.tile_pool(name="psO", bufs=2, space="PSUM") as psO,
        ):
            # ---- persistent constants / arrays ----------------------------
            ones_mat = pers.tile([P, P], R, tag="ones", name="ones_mat")
            nc.vector.memset(ones_mat[:, :], 1.0)
            bias_col = pers.tile([P, 1], F32, tag="bias", name="bias_col")
            # engine ops need 32-aligned start partitions: write the gap
            # as [64:96] then restore [64:77].
            nc.vector.memset(bias_col[:, :], 0.0)
            nc.vector.memset(bias_col[64:GAP1, :], -1e30)
            nc.vector.memset(bias_col[64:GAP0, :], 0.0)

            xkv_sb = pers.tile([P, NI * LP], R, tag="xkv", name="xkv_sb")
            kTMP = pers.tile([P, H], R, tag="kTMP", name="kTMP")
            v_sb = pers.tile([LP, HEADS * DH], R, tag="v", name="v_sb")
            kT_sb = [pers.tile([P, LP], R, tag=f"kT{j}", name=f"kT{j}") for j in range(NJ)]
            wq_sb = [pers.tile([P, NJ * P], R, tag=f"wq{j}", name=f"wq{j}") for j in range(NJ)]
            wout_sb = pers.tile([P, NJ * H], R, tag="wout", name="wout_sb")
            boutb_sb = pers.tile([P, H], F32, tag="boutb", name="boutb_sb")

            # ---- DMA prologue, in arrival-priority order ------------------
            nc.sync.dma_start(out=xkv_sb[:, :], in_=xkvp)
            hs_t = {}
            hs_t[0] = hsp.tile([P, NJ * 512], R, tag="hsT", name="hsT0")
            nc.sync.dma_start(out=hs_t[0][:, :], in_=hsTp[0])
            for j in range(NJ):
                nc.sync.dma_start(out=wq_sb[j][:, :], in_=wqp[j])
            kvh = []
            for ci in range(10):
                for hf in range(2):
                    t_ = kvwp.tile([P, 8 * 512], R, tag="kvw", name=f"kvw{ci}_{hf}")
                    nc.sync.dma_start(out=t_[:, :], in_=wkvh[ci, hf])
                    kvh.append(t_)
            hs_t[1] = hsp.tile([P, NJ * 512], R, tag="hsT", name="hsT1")
            nc.sync.dma_start(out=hs_t[1][:, :], in_=hsTp[1])
            nc.sync.dma_start(out=wout_sb[:, :], in_=woutp)
            nc.sync.dma_start(out=boutb_sb[:, :], in_=boutb)

            # ---- pipeline state -------------------------------------------
            pairs = [(c, hp) for c in range(NCH) for hp in range(NJ)]
            astate = {}
            qT_t = {}
            attnT_t = {}
            fin_t = {}

            def q_unit(c, j):
                ps = psA.tile([P, 512], F32, tag="acc", name="qps")
                for i in range(NJ):
                    nc.tensor.matmul(
                        ps[:, :], wq_sb[j][:, ts(i, P)], hs_t[c][:, ts(i, 512)],
                        start=(i == 0), stop=(i == NJ - 1),
                    )
                qt = qtp.tile([P, 512], R, tag=f"qT{j}", name=f"qT{j}")
                nc.scalar.copy(qt[:, :], ps[:, :])
                qT_t[(c, j)] = qt

            def kv_chunk(ci):
                proj, n = KV_PLAN[ci]
                ps = psA.tile([P, 512], F32, tag="acc", name="kvps")
                for i in range(NI):
                    src = kvh[2 * ci + (i // 8)]
                    nc.tensor.matmul(
                        ps[:, :], xkv_sb[:, ts(i, P)], src[:, ts(i % 8, 512)],
                        start=(i == 0), stop=(i == NI - 1),
                    )
                lo, hi = (0, P) if proj == 0 else (GAP1, P)
                if n < 2:
                    nc.scalar.copy(kTMP[lo:hi, ts(n, 512)], ps[lo:hi, :])
                elif n == 2:
                    nc.scalar.copy(kTMP[lo:hi, 1024:1280], ps[lo:hi, 0:256])
                    nc.scalar.copy(v_sb[lo:hi, 0:256], ps[lo:hi, 256:512])
                else:
                    v0 = 512 * n - 1280
                    nc.scalar.copy(v_sb[lo:hi, v0:v0 + 512], ps[lo:hi, :])
                if ci == 5:
                    # all k columns final -> transpose kTMP into kT (on the
                    # ACT hwdge queue so it doesn't queue behind SP triggers)
                    for j in range(NJ):
                        nc.scalar.dma_start(out=kT_sb[j][:, :],
                                            in_=kTMP[:, ts(j, P)], transpose=True)

            def attn_front(p):
                c, hp = pairs[p]
                probs = []
                for s_ in range(2):
                    rq = DH * s_
                    pss = psS.tile([P, 512], F32, tag="sps", name="sps")
                    nc.tensor.matmul(
                        pss[:, :], kT_sb[hp][rq:rq + DH, :],
                        qT_t[(c, hp)][rq:rq + DH, :],
                        start=True, stop=True,
                    )
                    pt = prp.tile([P, 512], R, tag="probsT", name="probsT")
                    nc.scalar.activation(pt[:, :], pss[:, :], EXP,
                                         bias=bias_col[:, :], scale=SCALE)
                    probs.append(pt)
                astate[p] = probs

            def attn_back(p):
                c, hp = pairs[p]
                probs = astate.pop(p)
                ps_o = psO.tile([P, 512], F32, tag="ops", name="ops")
                ps_d = psS.tile([P, 512], F32, tag="sps", name="dps")
                for s_ in range(2):
                    h = 2 * hp + s_
                    rq = DH * s_
                    nc.tensor.matmul(
                        ps_o[rq:rq + DH, :], v_sb[:, ts(h, DH)], probs[s_][:, :],
                        start=True, stop=True,
                    )
                    nc.tensor.matmul(
                        ps_d[rq:rq + DH, :], ones_mat[:, 0:DH], probs[s_][:, :],
                        start=True, stop=True,
                    )
                bc = bcp.tile([P, 512], F32, tag="bc", name="bc")
                nc.vector.reciprocal_approx_fast(bc[:, :], ps_d[:, :])
                at = atp.tile([P, 512], R, tag=f"attnT{hp}", name=f"attnT{hp}")
                nc.vector.tensor_mul(at[:, :], ps_o[:, :], bc[:, :])
                attnT_t[(c, hp)] = at

            def o_unit(c, u):
                tt, m = divmod(u, 3)
                m0 = m * 512
                mw = 512 if m < 2 else 256
                ps = psA.tile([P, 512], F32, tag="acc", name="ops2")
                for i in range(NJ):
                    nc.tensor.matmul(
                        ps[:, 0:mw], attnT_t[(c, i)][:, ts(tt, P)],
                        wout_sb[:, i * H + m0: i * H + m0 + mw],
                        start=(i == 0), stop=(i == NJ - 1),
                    )
                if m == 0:
                    fin_t[(c, tt)] = finp.tile([P, H], F32, tag="fin", name="fin")
                fin = fin_t[(c, tt)]
                nc.vector.tensor_add(fin[:, m0:m0 + mw], ps[:, 0:mw],
                                     boutb_sb[:, m0:m0 + mw])
                if m == 2:
                    nc.sync.dma_start(out=out[ts(4 * c + tt, P), :], in_=fin[:, :])

            # ---- the pipeline ---------------------------------------------
            for t in range(6):
                for j in range(NJ):
                    p = (t - 1) * NJ + j      # attention pair fronted here
                    pb = p - 2                # pair backed here (lookahead 2)
                    if 0 <= pb < NPAIR:
                        attn_back(pb)
                    if t < NCH:
                        q_unit(t, j)
                    if 0 <= p < NPAIR:
                        attn_front(p)
                    if t == 0:
                        kv_chunk(j)
                    # O-units start at j=2: attnT(co, 9) is only backed at
                    # j=1 of this chunk-slot (lookahead-2 attention backs)
                    co = t - 2
                    if 0 <= co < NCH and j >= 2:
                        for u in range(12 * (j - 2) // 8, 12 * (j - 1) // 8):
                            o_unit(co, u)
                    # late hsT chunks, issued inline so their WAR waits don't
                    # block the prologue DMA stream
                    if t == 0 and j == 6:
                        hs_t[2] = hsp.tile([P, NJ * 512], R, tag="hsT", name="hsT2")
                        nc.sync.dma_start(out=hs_t[2][:, :], in_=hsTp[2])
                    if t == 1 and j == 4:
                        hs_t[3] = hsp.tile([P, NJ * 512], R, tag="hsT", name="hsT3")
                        nc.sync.dma_start(out=hs_t[3][:, :], in_=hsTp[3])

    nc.compile()
    _NC_CACHE["nc"] = nc
    return nc


def prep_core_inputs(hidden_states, encoder_hidden_states, id_embedding,
                     Wq, Wk, Wv, Wid_k, Wid_v, Wout, bout):
    """Host-side sharding / layout prep.  Returns list of 8 in_maps."""
    f = np.float32
    h16 = np.float16
    hidden_states = np.asarray(hidden_states, f)
    encoder_hidden_states = np.asarray(encoder_hidden_states, f)
    id_embedding = np.asarray(id_embedding, f)
    Wq = np.asarray(Wq, f)
    Wout = np.asarray(Wout, f)
    Wk, Wv = np.asarray(Wk, f), np.asarray(Wv, f)
    Wid_k, Wid_v = np.asarray(Wid_k, f), np.asarray(Wid_v, f)
    boutb = np.ascontiguousarray(np.broadcast_to(np.asarray(bout, f), (P, H)))

    # packed batched-DMA weight layouts
    # wqp[j][p][i*128+r] = Wq[i*128+p, j*128+r]
    wqp = np.ascontiguousarray(
        Wq.reshape(NJ, P, NJ, P).transpose(2, 1, 0, 3).reshape(NJ, P, NJ * P)
        .astype(h16))

    def pack_kv(w):  # [C, 2560] -> [5, 2, P, 4096]
        a = w.reshape(NI, P, 5, 512)       # [i, p, n, q]
        a = a.transpose(2, 0, 1, 3)        # [n, i, p, q]
        a = a.reshape(5, 2, 8, P, 512)     # [n, h, i8, p, q]
        a = a.transpose(0, 1, 3, 2, 4)     # [n, h, p, i8, q]
        return a.reshape(5, 2, P, 4096)

    wkv5 = pack_kv(np.concatenate([Wk, Wv], axis=1))
    widkv5 = pack_kv(np.concatenate([Wid_k, Wid_v], axis=1))
    wkvh = np.ascontiguousarray(
        np.stack([(wkv5 if pr == 0 else widkv5)[n] for (pr, n) in KV_PLAN])
        .astype(h16))

    # woutp[p][i*H+m] = Wout[i*128+p, m]
    woutp = np.ascontiguousarray(
        Wout.reshape(NJ, P, H).transpose(1, 0, 2).reshape(P, NJ * H).astype(h16))

    in_maps = []
    for core in range(NCORES):
        b, hf = divmod(core, 2)
        hsT = hidden_states[b, hf * SC:(hf + 1) * SC, :].T  # [H, SC]
        # hsTp[c][p][i*512+q] = hsT[i*128+p, c*512+q]
        hsTp = np.ascontiguousarray(
            hsT.reshape(NJ, P, NCH, 512).transpose(2, 1, 0, 3)
            .reshape(NCH, P, NJ * 512).astype(h16))
        xkvT = np.zeros((C, LP), h16)
        xkvT[:, :TE] = encoder_hidden_states[b].T
        xkvT[:, GAP1:] = id_embedding[b % 2].T
        # xkvp[p][i*128+l] = xkvT[i*128+p, l]
        xkvp = np.ascontiguousarray(
            xkvT.reshape(NI, P, LP).transpose(1, 0, 2).reshape(P, NI * LP))
        in_maps.append({
            "hsTp": hsTp, "xkvp": xkvp, "wqp": wqp, "wkvh": wkvh,
            "woutp": woutp, "boutb": boutb,
        })
    return in_maps


def kernel(hidden_states, encoder_hidden_states, id_embedding,
           Wq, Wk, Wv, Wid_k, Wid_v, Wout, bout, _trace=False):
    _ensure_axon_hooks()
    from concourse.bass_utils import run_bass_kernel_spmd

    nc = build_nc()
    in_maps = prep_core_inputs(hidden_states, encoder_hidden_states, id_embedding,
                               Wq, Wk, Wv, Wid_k, Wid_v, Wout, bout)
    kwargs = {}
    if _trace:
        import concourse.bass_utils as bu
        bu.upload_artifacts = lambda tmpdir: f"local://{tmpdir}"
        kwargs["trace"] = True
    res = run_bass_kernel_spmd(nc, in_maps, core_ids=list(range(NCORES)), **kwargs)

    outp = np.empty((B, S, H), np.float32)
    for core in range(NCORES):
        b, hf = divmod(core, 2)
        outp[b, hf * SC:(hf + 1) * SC, :] = res.results[core]["out"]
    if _trace:
        kernel.last_exec_time_ns = res.exec_time_ns
        kernel.last_results = res
    return outp


# revision 8
# speedup vs baseline: 1.5604x; 1.0272x over previous
"""Trainium2 Bass kernel for the branched cross-attention processor.

Problem (full shapes):
  hidden_states [4, 4096, 1280], encoder_hidden_states [4, 77, 2048],
  id_embedding [2, 32, 2048], Wq/Wout [1280,1280], Wk/Wv/Wid_k/Wid_v
  [2048,1280], bout [1280].  20 heads, dh=64.  Output [4, 4096, 1280].

Sharding: data-parallel over (batch, seq-half): core c handles batch c//2,
query rows (c%2)*2048 : (c%2+1)*2048.  K/V (109 keys padded to 128:
[0:77]=ehs, [77:96]=zero gap, [96:128]=id) are computed per-core for its
batch.  No collectives.

Schedule: a 3-deep software pipeline over 4 query chunks of 512 keeps the
PE dense (TRN2 drops the PE clock from 2.4 to 1.2 GHz for ~3us after any
stall, so every bubble costs ~1.5us).  Chunk-slot t runs, interleaved at
head-pair granularity:
    Q-projection of chunk t | attention of chunk t-1 | out-proj of t-2
The kv projection (10 weight chunks) fills chunk-slot 0.  Attention
per pair: scoresT = kT^T qT -> exp (ACT, gap-masked bias) -> PV + ones
matmul denominator (PE) -> reciprocal+normalize (DVE).  The exp/recip/mul
engine work hides under the Q/O matmuls of the same slot.

DMAs are batched into ~60 large transfers (the per-dma_start trigger is
~0.6us of SP sequencer time) and issued in arrival-priority order.
"""

import os
import sys
import types

import numpy as np

# ---------------------------------------------------------------------------
# problem constants (hardcoded; kernel.py must be self-contained)
# ---------------------------------------------------------------------------
B = 4
S = 4096
H = 1280
C = 2048
TE = 77          # encoder tokens
TI = 32          # id tokens
HEADS = 20
DH = 64          # head dim
P = 128
L = 109          # TE + TI
LP = 128         # padded key count
GAP0, GAP1 = TE, P - TI   # 77, 96
SC = 2048        # seq rows per core
NJ = H // P      # 10
NI = C // P      # 16
NCH = SC // 512  # 4 query chunks of 512
NT = SC // P     # 16 q-tiles of 128
SCALE = 1.0 / 8.0
NCORES = 8
NPAIR = NCH * NJ  # 40 (chunk, head-pair) attention units
# kv chunk plan: proj 0 = [Wk|Wv] (ehs rows), proj 1 = [Wid_k|Wid_v]
# (id rows).  k columns first so kT transposes can fire at index 5.
KV_PLAN = [(0, 0), (0, 1), (0, 2), (1, 0), (1, 1), (1, 2),
           (0, 3), (1, 3), (0, 4), (1, 4)]

_NC_CACHE = {}


def _ensure_axon_hooks():
    """The image's antenv lacks axon_hooks; synthesize it so NTFF profiling
    (trace=True) works when test.py asks for it.  Harmless if unused."""
    if "antenv.axon_hooks" in sys.modules:
        return
    try:
        import antenv
        from trn_agent_boot.trn_boot import _ntff_profile_via_ctypes

        hook = _ntff_profile_via_ctypes("/opt/axon/libaxon_pjrt.so")
        m = types.ModuleType("antenv.axon_hooks")
        m.get_axon_ntff_profile_hook = lambda: hook
        m.set_axon_ntff_profile_hook = lambda h: None
        sys.modules["antenv.axon_hooks"] = m
        antenv.axon_hooks = m
    except Exception:
        pass


def build_nc():
    """Build + compile the per-core Bass program (SPMD: same NEFF, 8 cores)."""
    if "nc" in _NC_CACHE:
        return _NC_CACHE["nc"]

    import concourse.bass as bass
    import concourse.tile as tile
    from concourse import bacc, mybir
    from concourse.bass import ts

    F32 = mybir.dt.float32
    R = mybir.dt.float16      # matmul operand dtype (1 cyc/row)
    EXP = mybir.ActivationFunctionType.Exp

    nc = bacc.Bacc("TRN2", target_bir_lowering=False, debug=False, num_devices=NCORES)

    hsTp = nc.dram_tensor("hsTp", [NCH, P, NJ * 512], R, kind="ExternalInput").ap()
    xkvp = nc.dram_tensor("xkvp", [P, NI * LP], R, kind="ExternalInput").ap()
    wqp = nc.dram_tensor("wqp", [NJ, P, NJ * P], R, kind="ExternalInput").ap()
    wkvh = nc.dram_tensor("wkvh", [10, 2, P, 8 * 512], R, kind="ExternalInput").ap()
    woutp = nc.dram_tensor("woutp", [P, NJ * H], R, kind="ExternalInput").ap()
    boutb = nc.dram_tensor("boutb", [P, H], F32, kind="ExternalInput").ap()
    out = nc.dram_tensor("out", [SC, H], F32, kind="ExternalOutput").ap()

    with tile.TileContext(nc) as tc:
        with (
            tc.tile_pool(name="pers", bufs=1) as pers,
            tc.tile_pool(name="hsp", bufs=2) as hsp,
            tc.tile_pool(name="qtp", bufs=2) as qtp,
            tc.tile_pool(name="atp", bufs=2) as atp,
            tc.tile_pool(name="kvwp", bufs=4) as kvwp,
            tc.tile_pool(name="prp", bufs=6) as prp,
            tc.tile_pool(name="bcp", bufs=2) as bcp,
            tc.tile_pool(name="finp", bufs=3) as finp,
            tc.tile_pool(name="psA", bufs=3, space="PSUM") as psA,
            tc.tile_pool(name="psS", bufs=3, space="PSUM") as psS,
            tc.tile_pool(name="psO", bufs=2, space="PSUM") as psO,
        ):
            # ---- persistent constants / arrays ----------------------------
            ones_mat = pers.tile([P, P], R, tag="ones", name="ones_mat")
            nc.vector.memset(ones_mat[:, :], 1.0)
            bias_col = pers.tile([P, 1], F32, tag="bias", name="bias_col")
            # engine ops need 32-aligned start partitions: write the gap
            # as [64:96] then restore [64:77].
            nc.vector.memset(bias_col[:, :], 0.0)
            nc.vector.memset(bias_col[64:GAP1, :], -1e30)
            nc.vector.memset(bias_col[64:GAP0, :], 0.0)

            xkv_sb = pers.tile([P, NI * LP], R, tag="xkv", name="xkv_sb")
            kTMP = pers.tile([P, H], R, tag="kTMP", name="kTMP")
            v_sb = pers.tile([LP, HEADS * DH], R, tag="v", name="v_sb")
            kT_sb = [pers.tile([P, LP], R, tag=f"kT{j}", name=f"kT{j}") for j in range(NJ)]
            wq_sb = [pers.tile([P, NJ * P], R, tag=f"wq{j}", name=f"wq{j}") for j in range(NJ)]
            wout_sb = pers.tile([P, NJ * H], R, tag="wout", name="wout_sb")
            boutb_sb = pers.tile([P, H], F32, tag="boutb", name="boutb_sb")

            # ---- DMA prologue, in arrival-priority order ------------------
            nc.sync.dma_start(out=xkv_sb[:, :], in_=xkvp)
            hs_t = {}
            hs_t[0] = hsp.tile([P, NJ * 512], R, tag="hsT", name="hsT0")
            nc.sync.dma_start(out=hs_t[0][:, :], in_=hsTp[0])
            for j in range(NJ):
                nc.sync.dma_start(out=wq_sb[j][:, :], in_=wqp[j])
            kvh = []

            def kv_dma(ci):
                for hf in range(2):
                    t_ = kvwp.tile([P, 8 * 512], R, tag="kvw", name=f"kvw{ci}_{hf}")
                    nc.sync.dma_start(out=t_[:, :], in_=wkvh[ci, hf])
                    kvh.append(t_)

            for ci in range(6):          # k-chunk weights (chunk-slot 0)
                kv_dma(ci)
            hs_t[1] = hsp.tile([P, NJ * 512], R, tag="hsT", name="hsT1")
            nc.sync.dma_start(out=hs_t[1][:, :], in_=hsTp[1])
            for ci in range(6, 10):      # v-chunk weights (chunk-slot 1)
                kv_dma(ci)
            nc.sync.dma_start(out=wout_sb[:, :], in_=woutp)
            nc.sync.dma_start(out=boutb_sb[:, :], in_=boutb)

            # ---- pipeline state -------------------------------------------
            pairs = [(c, hp) for c in range(NCH) for hp in range(NJ)]
            astate = {}
            qT_t = {}
            attnT_t = {}
            fin_t = {}

            def q_unit(c, j):
                ps = psA.tile([P, 512], F32, tag="acc", name="qps")
                for i in range(NJ):
                    nc.tensor.matmul(
                        ps[:, :], wq_sb[j][:, ts(i, P)], hs_t[c][:, ts(i, 512)],
                        start=(i == 0), stop=(i == NJ - 1),
                    )
                qt = qtp.tile([P, 512], R, tag=f"qT{j}", name=f"qT{j}")
                nc.scalar.copy(qt[:, :], ps[:, :])
                qT_t[(c, j)] = qt

            def kv_chunk(ci):
                proj, n = KV_PLAN[ci]
                ps = psA.tile([P, 512], F32, tag="acc", name="kvps")
                for i in range(NI):
                    src = kvh[2 * ci + (i // 8)]
                    nc.tensor.matmul(
                        ps[:, :], xkv_sb[:, ts(i, P)], src[:, ts(i % 8, 512)],
                        start=(i == 0), stop=(i == NI - 1),
                    )
                # copies on the DVE (idle during the fill phase; GPSIMD has
                # no PSUM port) so the in-order ACT queue (qT copies + exp)
                # never waits behind DMA-paced kv chunks
                lo, hi = (0, P) if proj == 0 else (GAP1, P)
                if n < 2:
                    nc.vector.tensor_scalar_add(kTMP[lo:hi, ts(n, 512)], ps[lo:hi, :], 0.0)
                elif n == 2:
                    nc.vector.tensor_scalar_add(kTMP[lo:hi, 1024:1280], ps[lo:hi, 0:256], 0.0)
                    nc.vector.tensor_scalar_add(v_sb[lo:hi, 0:256], ps[lo:hi, 256:512], 0.0)
                else:
                    v0 = 512 * n - 1280
                    nc.vector.tensor_scalar_add(v_sb[lo:hi, v0:v0 + 512], ps[lo:hi, :], 0.0)
                # k column ranges finalize per (1, n) chunk: transpose each
                # kT block as soon as both projections have written it (on
                # the ACT hwdge queue so SP's paced kv triggers don't delay)
                KT_BATCH = {3: range(0, 4), 4: range(4, 8), 5: range(8, NJ)}
                if ci in KT_BATCH:
                    for j in KT_BATCH[ci]:
                        nc.scalar.dma_start(out=kT_sb[j][:, :],
                                            in_=kTMP[:, ts(j, P)], transpose=True)

            def attn_front(p):
                c, hp = pairs[p]
                probs = []
                for s_ in range(2):
                    rq = DH * s_
                    pss = psS.tile([P, 512], F32, tag="sps", name="sps")
                    nc.tensor.matmul(
                        pss[:, :], kT_sb[hp][rq:rq + DH, :],
                        qT_t[(c, hp)][rq:rq + DH, :],
                        start=True, stop=True,
                    )
                    pt = prp.tile([P, 512], R, tag="probsT", name="probsT")
                    nc.scalar.activation(pt[:, :], pss[:, :], EXP,
                                         bias=bias_col[:, :], scale=SCALE)
                    probs.append(pt)
                astate[p] = probs

            def attn_back(p):
                c, hp = pairs[p]
                probs = astate.pop(p)
                ps_o = psO.tile([P, 512], F32, tag="ops", name="ops")
                ps_d = psS.tile([P, 512], F32, tag="sps", name="dps")
                for s_ in range(2):
                    h = 2 * hp + s_
                    rq = DH * s_
                    nc.tensor.matmul(
                        ps_o[rq:rq + DH, :], v_sb[:, ts(h, DH)], probs[s_][:, :],
                        start=True, stop=True,
                    )
                    nc.tensor.matmul(
                        ps_d[rq:rq + DH, :], ones_mat[:, 0:DH], probs[s_][:, :],
                        start=True, stop=True,
                    )
                bc = bcp.tile([P, 512], F32, tag="bc", name="bc")
                nc.vector.reciprocal_approx_fast(bc[:, :], ps_d[:, :])
                at = atp.tile([P, 512], R, tag=f"attnT{hp}", name=f"attnT{hp}")
                nc.vector.tensor_mul(at[:, :], ps_o[:, :], bc[:, :])
                attnT_t[(c, hp)] = at

            def o_unit(c, u):
                tt, m = divmod(u, 3)
                m0 = m * 512
                mw = 512 if m < 2 else 256
                ps = psA.tile([P, 512], F32, tag="acc", name="ops2")
                for i in range(NJ):
                    nc.tensor.matmul(
                        ps[:, 0:mw], attnT_t[(c, i)][:, ts(tt, P)],
                        wout_sb[:, i * H + m0: i * H + m0 + mw],
                        start=(i == 0), stop=(i == NJ - 1),
                    )
                if m == 0:
                    fin_t[(c, tt)] = finp.tile([P, H], F32, tag="fin", name="fin")
                fin = fin_t[(c, tt)]
                nc.vector.tensor_add(fin[:, m0:m0 + mw], ps[:, 0:mw],
                                     boutb_sb[:, m0:m0 + mw])
                if m == 2:
                    nc.sync.dma_start(out=out[ts(4 * c + tt, P), :], in_=fin[:, :])

            # ---- the pipeline ---------------------------------------------
            for t in range(6):
                for j in range(NJ):
                    p = (t - 1) * NJ + j      # attention pair fronted here
                    pb = p - 2                # pair backed here (lookahead 2)
                    if 0 <= pb < NPAIR:
                        attn_back(pb)
                    if t < NCH:
                        q_unit(t, j)
                    if 0 <= p < NPAIR:
                        attn_front(p)
                    # kv chunks placed to match DMA arrival: the 6 k-chunks
                    # fill chunk-slot 0 slots 4-9 (Q(0) runs first while the
                    # kv weight stream is still in flight); the 4 v-chunks
                    # land in chunk-slot 1 slots 1/3/5/7, just ahead of the
                    # attention backs that read each v column range.
                    if t == 0 and j >= 4:
                        kv_chunk(j - 4)
                    if t == 1 and j in (1, 3, 5, 7):
                        kv_chunk(6 + (j - 1) // 2)
                    # O-units start at j=2: attnT(co, 9) is only backed at
                    # j=1 of this chunk-slot (lookahead-2 attention backs)
                    co = t - 2
                    if 0 <= co < NCH and j >= 2:
                        for u in range(12 * (j - 2) // 8, 12 * (j - 1) // 8):
                            o_unit(co, u)
                    # late hsT chunks, issued inline so their WAR waits don't
                    # block the prologue DMA stream
                    if t == 0 and j == 6:
                        hs_t[2] = hsp.tile([P, NJ * 512], R, tag="hsT", name="hsT2")
                        nc.sync.dma_start(out=hs_t[2][:, :], in_=hsTp[2])
                    if t == 1 and j == 4:
                        hs_t[3] = hsp.tile([P, NJ * 512], R, tag="hsT", name="hsT3")
                        nc.sync.dma_start(out=hs_t[3][:, :], in_=hsTp[3])

    nc.compile()
    _NC_CACHE["nc"] = nc
    return nc


def prep_core_inputs(hidden_states, encoder_hidden_states, id_embedding,
                     Wq, Wk, Wv, Wid_k, Wid_v, Wout, bout):
    """Host-side sharding / layout prep.  Returns list of 8 in_maps."""
    f = np.float32
    h16 = np.float16
    hidden_states = np.asarray(hidden_states, f)
    encoder_hidden_states = np.asarray(encoder_hidden_states, f)
    id_embedding = np.asarray(id_embedding, f)
    Wq = np.asarray(Wq, f)
    Wout = np.asarray(Wout, f)
    Wk, Wv = np.asarray(Wk, f), np.asarray(Wv, f)
    Wid_k, Wid_v = np.asarray(Wid_k, f), np.asarray(Wid_v, f)
    boutb = np.ascontiguousarray(np.broadcast_to(np.asarray(bout, f), (P, H)))

    # packed batched-DMA weight layouts
    # wqp[j][p][i*128+r] = Wq[i*128+p, j*128+r]
    wqp = np.ascontiguousarray(
        Wq.reshape(NJ, P, NJ, P).transpose(2, 1, 0, 3).reshape(NJ, P, NJ * P)
        .astype(h16))

    def pack_kv(w):  # [C, 2560] -> [5, 2, P, 4096]
        a = w.reshape(NI, P, 5, 512)       # [i, p, n, q]
        a = a.transpose(2, 0, 1, 3)        # [n, i, p, q]
        a = a.reshape(5, 2, 8, P, 512)     # [n, h, i8, p, q]
        a = a.transpose(0, 1, 3, 2, 4)     # [n, h, p, i8, q]
        return a.reshape(5, 2, P, 4096)

    wkv5 = pack_kv(np.concatenate([Wk, Wv], axis=1))
    widkv5 = pack_kv(np.concatenate([Wid_k, Wid_v], axis=1))
    wkvh = np.ascontiguousarray(
        np.stack([(wkv5 if pr == 0 else widkv5)[n] for (pr, n) in KV_PLAN])
        .astype(h16))

    # woutp[p][i*H+m] = Wout[i*128+p, m]
    woutp = np.ascontiguousarray(
        Wout.reshape(NJ, P, H).transpose(1, 0, 2).reshape(P, NJ * H).astype(h16))

    in_maps = []
    for core in range(NCORES):
        b, hf = divmod(core, 2)
        hsT = hidden_states[b, hf * SC:(hf + 1) * SC, :].T  # [H, SC]
        # hsTp[c][p][i*512+q] = hsT[i*128+p, c*512+q]
        hsTp = np.ascontiguousarray(
            hsT.reshape(NJ, P, NCH, 512).transpose(2, 1, 0, 3)
            .reshape(NCH, P, NJ * 512).astype(h16))
        xkvT = np.zeros((C, LP), h16)
        xkvT[:, :TE] = encoder_hidden_states[b].T
        xkvT[:, GAP1:] = id_embedding[b % 2].T
        # xkvp[p][i*128+l] = xkvT[i*128+p, l]
        xkvp = np.ascontiguousarray(
            xkvT.reshape(NI, P, LP).transpose(1, 0, 2).reshape(P, NI * LP))
        in_maps.append({
            "hsTp": hsTp, "xkvp": xkvp, "wqp": wqp, "wkvh": wkvh,
            "woutp": woutp, "boutb": boutb,
        })
    return in_maps


def kernel(hidden_states, encoder_hidden_states, id_embedding,
           Wq, Wk, Wv, Wid_k, Wid_v, Wout, bout, _trace=False):
    _ensure_axon_hooks()
    from concourse.bass_utils import run_bass_kernel_spmd

    nc = build_nc()
    in_maps = prep_core_inputs(hidden_states, encoder_hidden_states, id_embedding,
                               Wq, Wk, Wv, Wid_k, Wid_v, Wout, bout)
    kwargs = {}
    if _trace:
        import concourse.bass_utils as bu
        bu.upload_artifacts = lambda tmpdir: f"local://{tmpdir}"
        kwargs["trace"] = True
    res = run_bass_kernel_spmd(nc, in_maps, core_ids=list(range(NCORES)), **kwargs)

    outp = np.empty((B, S, H), np.float32)
    for core in range(NCORES):
        b, hf = divmod(core, 2)
        outp[b, hf * SC:(hf + 1) * SC, :] = res.results[core]["out"]
    if _trace:
        kernel.last_exec_time_ns = res.exec_time_ns
        kernel.last_results = res
    return outp


# revision 19
# speedup vs baseline: 1.7144x; 1.0987x over previous
"""Trainium2 Bass kernel for the branched cross-attention processor.

Problem (full shapes):
  hidden_states [4, 4096, 1280], encoder_hidden_states [4, 77, 2048],
  id_embedding [2, 32, 2048], Wq/Wout [1280,1280], Wk/Wv/Wid_k/Wid_v
  [2048,1280], bout [1280].  20 heads, dh=64.  Output [4, 4096, 1280].

Sharding: data-parallel over (batch, seq-half): core c handles batch c//2,
query rows (c%2)*2048 : (c%2+1)*2048.  K/V (109 keys padded to 128:
[0:77]=ehs, [77:96]=zero gap, [96:128]=id) are computed per-core for its
batch.  No collectives.

Schedule: a 3-deep software pipeline over 4 query chunks of 512 keeps the
PE dense (TRN2 drops the PE clock from 2.4 to 1.2 GHz for ~3us after any
stall, so every bubble costs ~1.5us).  Chunk-slot t runs, interleaved at
head-pair granularity:
    Q-projection of chunk t | attention of chunk t-1 | out-proj of t-2
The kv projection (10 weight chunks) fills chunk-slot 0.  Attention
per pair: scoresT = kT^T qT -> exp (ACT, gap-masked bias) -> PV + ones
matmul denominator (PE) -> reciprocal+normalize (DVE).  The exp/recip/mul
engine work hides under the Q/O matmuls of the same slot.

DMAs are batched into ~60 large transfers (the per-dma_start trigger is
~0.6us of SP sequencer time) and issued in arrival-priority order.
"""

import os
import sys
import types

import numpy as np

# ---------------------------------------------------------------------------
# problem constants (hardcoded; kernel.py must be self-contained)
# ---------------------------------------------------------------------------
B = 4
S = 4096
H = 1280
C = 2048
TE = 77          # encoder tokens
TI = 32          # id tokens
HEADS = 20
DH = 64          # head dim
P = 128
L = 109          # TE + TI
LP = 128         # padded key count
GAP0, GAP1 = TE, P - TI   # 77, 96
SC = 2048        # seq rows per core
NJ = H // P      # 10
NI = C // P      # 16
NCH = SC // 512  # 4 query chunks of 512
NT = SC // P     # 16 q-tiles of 128
SCALE = 1.0 / 8.0
NCORES = 8
NPAIR = NCH * NJ  # 40 (chunk, head-pair) attention units
# kv chunk plan: proj 0 = [Wk|Wv] (ehs rows), proj 1 = [Wid_k|Wid_v]
# (id rows).  k columns first so kT transposes can fire at index 5.
KV_PLAN = [(0, 0), (0, 1), (0, 2), (1, 0), (1, 1), (1, 2),
           (0, 3), (1, 3), (0, 4), (1, 4)]

_NC_CACHE = {}


def _ensure_axon_hooks():
    """The image's antenv lacks axon_hooks; synthesize it so NTFF profiling
    (trace=True) works when test.py asks for it.  Harmless if unused."""
    if "antenv.axon_hooks" in sys.modules:
        return
    try:
        import antenv
        from trn_agent_boot.trn_boot import _ntff_profile_via_ctypes

        hook = _ntff_profile_via_ctypes("/opt/axon/libaxon_pjrt.so")
        m = types.ModuleType("antenv.axon_hooks")
        m.get_axon_ntff_profile_hook = lambda: hook
        m.set_axon_ntff_profile_hook = lambda h: None
        sys.modules["antenv.axon_hooks"] = m
        antenv.axon_hooks = m
    except Exception:
        pass


def build_nc():
    """Build + compile the per-core Bass program (SPMD: same NEFF, 8 cores)."""
    if "nc" in _NC_CACHE:
        return _NC_CACHE["nc"]

    import concourse.bass as bass
    import concourse.tile as tile
    from concourse import bacc, mybir
    from concourse.bass import ts

    F32 = mybir.dt.float32
    R = mybir.dt.float16      # matmul operand dtype (1 cyc/row)
    EXP = mybir.ActivationFunctionType.Exp

    nc = bacc.Bacc("TRN2", target_bir_lowering=False, debug=False, num_devices=NCORES)

    ident = nc.dram_tensor("ident", [P, P], R, kind="ExternalInput").ap()
    hsTp = nc.dram_tensor("hsTp", [NCH, P, NJ * 512], R, kind="ExternalInput").ap()
    xkvp = nc.dram_tensor("xkvp", [P, NI * LP], R, kind="ExternalInput").ap()
    wqp = nc.dram_tensor("wqp", [NJ, P, NJ * P], R, kind="ExternalInput").ap()
    wkvh = nc.dram_tensor("wkvh", [10, 2, P, 8 * 512], R, kind="ExternalInput").ap()
    woutp = nc.dram_tensor("woutp", [P, NJ * H], R, kind="ExternalInput").ap()
    boutb = nc.dram_tensor("boutb", [P, H], F32, kind="ExternalInput").ap()
    out = nc.dram_tensor("out", [SC, H], F32, kind="ExternalOutput").ap()

    with tile.TileContext(nc) as tc:
        with (
            tc.tile_pool(name="pers", bufs=1) as pers,
            tc.tile_pool(name="hsp", bufs=2) as hsp,
            tc.tile_pool(name="qtp", bufs=2) as qtp,
            tc.tile_pool(name="atp", bufs=2) as atp,
            tc.tile_pool(name="kvwp", bufs=6) as kvwp,
            tc.tile_pool(name="prp", bufs=6) as prp,
            tc.tile_pool(name="bcp", bufs=2) as bcp,
            tc.tile_pool(name="finp", bufs=2) as finp,
            tc.tile_pool(name="psA", bufs=3, space="PSUM") as psA,
            tc.tile_pool(name="psS", bufs=3, space="PSUM") as psS,
            tc.tile_pool(name="psO", bufs=2, space="PSUM") as psO,
        ):
            # ---- persistent constants / arrays ----------------------------
            ones_mat = pers.tile([P, P], R, tag="ones", name="ones_mat")
            nc.vector.memset(ones_mat[:, :], 1.0)
            bias_col = pers.tile([P, 1], F32, tag="bias", name="bias_col")
            # engine ops need 32-aligned start partitions: write the gap
            # as [64:96] then restore [64:77].
            nc.vector.memset(bias_col[:, :], 0.0)
            nc.vector.memset(bias_col[64:GAP1, :], -1e30)
            nc.vector.memset(bias_col[64:GAP0, :], 0.0)

            ident_sb = pers.tile([P, P], R, tag="ident", name="ident_sb")
            xkv_sb = pers.tile([P, NI * LP], R, tag="xkv", name="xkv_sb")
            kTMP = pers.tile([P, H], R, tag="kTMP", name="kTMP")
            v_sb = pers.tile([LP, HEADS * DH], R, tag="v", name="v_sb")
            kT_sb = [pers.tile([P, LP], R, tag=f"kT{j}", name=f"kT{j}") for j in range(NJ)]
            wq_sb = [pers.tile([P, NJ * P], R, tag=f"wq{j}", name=f"wq{j}") for j in range(NJ)]
            wout_sb = pers.tile([P, NJ * H], R, tag="wout", name="wout_sb")
            boutb_sb = pers.tile([P, H], F32, tag="boutb", name="boutb_sb")

            # ---- DMA prologue, in arrival-priority order ------------------
            nc.sync.dma_start(out=ident_sb[:, :], in_=ident)
            nc.sync.dma_start(out=xkv_sb[:, :], in_=xkvp)
            hs_t = {}
            hs_t[0] = hsp.tile([P, NJ * 512], R, tag="hsT", name="hsT0")
            nc.sync.dma_start(out=hs_t[0][:, :], in_=hsTp[0])
            for j in range(NJ):
                nc.sync.dma_start(out=wq_sb[j][:, :], in_=wqp[j])
            kvh = []

            def kv_dma(ci):
                for hf in range(2):
                    t_ = kvwp.tile([P, 8 * 512], R, tag="kvw", name=f"kvw{ci}_{hf}")
                    nc.sync.dma_start(out=t_[:, :], in_=wkvh[ci, hf])
                    kvh.append(t_)

            for ci in range(4):          # k-chunk weights (chunk-slot 0)
                kv_dma(ci)
            hs_t[1] = hsp.tile([P, NJ * 512], R, tag="hsT", name="hsT1")
            nc.sync.dma_start(out=hs_t[1][:, :], in_=hsTp[1])
            for ci in range(4, 10):      # rest of k + v weights
                kv_dma(ci)
            nc.sync.dma_start(out=wout_sb[:, :], in_=woutp)
            nc.sync.dma_start(out=boutb_sb[:, :], in_=boutb)

            # ---- pipeline state -------------------------------------------
            pairs = [(c, hp) for c in range(NCH) for hp in range(NJ)]
            astate = {}
            qT_t = {}
            attnT_t = {}
            fin_t = {}

            def q_unit(c, j):
                ps = psA.tile([P, 512], F32, tag="acc", name="qps")
                for i in range(NJ):
                    nc.tensor.matmul(
                        ps[:, :], wq_sb[j][:, ts(i, P)], hs_t[c][:, ts(i, 512)],
                        start=(i == 0), stop=(i == NJ - 1),
                    )
                qt = qtp.tile([P, 512], R, tag=f"qT{j}", name=f"qT{j}")
                nc.scalar.copy(qt[:, :], ps[:, :])
                qT_t[(c, j)] = qt

            def kv_chunk(ci):
                proj, n = KV_PLAN[ci]
                # psO is idle during the fill chunk-slots; using it keeps the
                # kv chain off the Q-copy-paced psA rotation
                ps = psO.tile([P, 512], F32, tag="ops", name="kvps")
                for i in range(NI):
                    src = kvh[2 * ci + (i // 8)]
                    nc.tensor.matmul(
                        ps[:, :], xkv_sb[:, ts(i, P)], src[:, ts(i % 8, 512)],
                        start=(i == 0), stop=(i == NI - 1),
                    )
                # copies on the DVE (idle during the fill phase; GPSIMD has
                # no PSUM port) so the in-order ACT queue (qT copies + exp)
                # never waits behind DMA-paced kv chunks
                lo, hi = (0, P) if proj == 0 else (GAP1, P)
                if n < 2:
                    nc.vector.tensor_scalar_add(kTMP[lo:hi, ts(n, 512)], ps[lo:hi, :], 0.0)
                elif n == 2:
                    nc.vector.tensor_scalar_add(kTMP[lo:hi, 1024:1280], ps[lo:hi, 0:256], 0.0)
                    nc.vector.tensor_scalar_add(v_sb[lo:hi, 0:256], ps[lo:hi, 256:512], 0.0)
                else:
                    v0 = 512 * n - 1280
                    nc.vector.tensor_scalar_add(v_sb[lo:hi, v0:v0 + 512], ps[lo:hi, :], 0.0)
                # k column ranges finalize per (1, n) chunk: transpose each
                # kT block as soon as both projections have written it.  PE
                # transposes (~0.1us each) instead of DMA transposes: the
                # latter cost 1.2us apiece on the ACT hwdge queue and starve
                # the first exps.
                KT_BATCH = {3: range(0, 4), 4: range(4, 8), 5: range(8, NJ)}
                if ci in KT_BATCH:
                    for j in KT_BATCH[ci]:
                        tps = psO.tile([P, P], R, tag="ops", name="tps")
                        nc.tensor.transpose(tps[:, :], kTMP[:, ts(j, P)], ident_sb[:, :])
                        nc.vector.tensor_copy(kT_sb[j][:, :], tps[:, :])

            def attn_front(p):
                c, hp = pairs[p]
                probs = []
                for s_ in range(2):
                    rq = DH * s_
                    pss = psS.tile([P, 512], F32, tag="sps", name="sps")
                    nc.tensor.matmul(
                        pss[:, :], kT_sb[hp][rq:rq + DH, :],
                        qT_t[(c, hp)][rq:rq + DH, :],
                        start=True, stop=True,
                    )
                    pt = prp.tile([P, 512], R, tag="probsT", name="probsT")
                    nc.scalar.activation(pt[:, :], pss[:, :], EXP,
                                         bias=bias_col[:, :], scale=SCALE)
                    probs.append(pt)
                astate[p] = probs

            def attn_back(p):
                c, hp = pairs[p]
                probs = astate.pop(p)
                ps_o = psO.tile([P, 512], F32, tag="ops", name="ops")
                ps_d = psS.tile([P, 512], F32, tag="sps", name="dps")
                for s_ in range(2):
                    h = 2 * hp + s_
                    rq = DH * s_
                    nc.tensor.matmul(
                        ps_o[rq:rq + DH, :], v_sb[:, ts(h, DH)], probs[s_][:, :],
                        start=True, stop=True,
                    )
                    nc.tensor.matmul(
                        ps_d[rq:rq + DH, :], ones_mat[:, 0:DH], probs[s_][:, :],
                        start=True, stop=True,
                    )
                bc = bcp.tile([P, 512], F32, tag="bc", name="bc")
                nc.vector.reciprocal_approx_fast(bc[:, :], ps_d[:, :])
                at = atp.tile([P, 512], R, tag=f"attnT{hp}", name=f"attnT{hp}")
                nc.vector.tensor_mul(at[:, :], ps_o[:, :], bc[:, :])
                attnT_t[(c, hp)] = at

            def o_unit(c, u):
                tt, m = divmod(u, 3)
                m0 = m * 512
                mw = 512 if m < 2 else 256
                ps = psA.tile([P, 512], F32, tag="acc", name="ops2")
                for i in range(NJ):
                    nc.tensor.matmul(
                        ps[:, 0:mw], attnT_t[(c, i)][:, ts(tt, P)],
                        wout_sb[:, i * H + m0: i * H + m0 + mw],
                        start=(i == 0), stop=(i == NJ - 1),
                    )
                if m == 0:
                    fin_t[(c, tt)] = finp.tile([P, H], F32, tag="fin", name="fin")
                fin = fin_t[(c, tt)]
                nc.vector.tensor_add(fin[:, m0:m0 + mw], ps[:, 0:mw],
                                     boutb_sb[:, m0:m0 + mw])
                if m == 2:
                    nc.sync.dma_start(out=out[ts(4 * c + tt, P), :], in_=fin[:, :])

            # ---- the pipeline ---------------------------------------------
            for t in range(6):
                for j in range(NJ):
                    p = (t - 1) * NJ + j      # attention pair fronted here
                    pb = p - 2                # pair backed here (lookahead 2)
                    if 0 <= pb < NPAIR:
                        attn_back(pb)
                    if t < NCH:
                        q_unit(t, j)
                    if 0 <= p < NPAIR:
                        attn_front(p)
                    # kv chunks placed to match DMA arrival: the 6 k-chunks
                    # fill chunk-slot 0 slots 4-9 (Q(0) runs first while the
                    # kv weight stream is still in flight); the 4 v-chunks
                    # land in chunk-slot 1 slots 1/3/5/7, just ahead of the
                    # attention backs that read each v column range.
                    if t == 0 and j >= 4:
                        kv_chunk(j - 4)
                    if t == 1 and j in (1, 3, 5, 7):
                        kv_chunk(6 + (j - 1) // 2)
                    # O-units start at j=2: attnT(co, 9) is only backed at
                    # j=1 of this chunk-slot (lookahead-2 attention backs)
                    co = t - 2
                    if 0 <= co < NCH and j >= 2:
                        for u in range(12 * (j - 2) // 8, 12 * (j - 1) // 8):
                            o_unit(co, u)
                    # late hsT chunks, issued inline so their WAR waits don't
                    # block the prologue DMA stream
                    if t == 0 and j == 6:
                        hs_t[2] = hsp.tile([P, NJ * 512], R, tag="hsT", name="hsT2")
                        nc.sync.dma_start(out=hs_t[2][:, :], in_=hsTp[2])
                    if t == 1 and j == 4:
                        hs_t[3] = hsp.tile([P, NJ * 512], R, tag="hsT", name="hsT3")
                        nc.sync.dma_start(out=hs_t[3][:, :], in_=hsTp[3])

    nc.compile()
    _NC_CACHE["nc"] = nc
    return nc


def prep_core_inputs(hidden_states, encoder_hidden_states, id_embedding,
                     Wq, Wk, Wv, Wid_k, Wid_v, Wout, bout):
    """Host-side sharding / layout prep.  Returns list of 8 in_maps."""
    f = np.float32
    h16 = np.float16
    hidden_states = np.asarray(hidden_states, f)
    encoder_hidden_states = np.asarray(encoder_hidden_states, f)
    id_embedding = np.asarray(id_embedding, f)
    Wq = np.asarray(Wq, f)
    Wout = np.asarray(Wout, f)
    Wk, Wv = np.asarray(Wk, f), np.asarray(Wv, f)
    Wid_k, Wid_v = np.asarray(Wid_k, f), np.asarray(Wid_v, f)
    boutb = np.ascontiguousarray(np.broadcast_to(np.asarray(bout, f), (P, H)))

    # packed batched-DMA weight layouts
    # wqp[j][p][i*128+r] = Wq[i*128+p, j*128+r]
    wqp = np.ascontiguousarray(
        Wq.reshape(NJ, P, NJ, P).transpose(2, 1, 0, 3).reshape(NJ, P, NJ * P)
        .astype(h16))

    def pack_kv(w):  # [C, 2560] -> [5, 2, P, 4096]
        a = w.reshape(NI, P, 5, 512)       # [i, p, n, q]
        a = a.transpose(2, 0, 1, 3)        # [n, i, p, q]
        a = a.reshape(5, 2, 8, P, 512)     # [n, h, i8, p, q]
        a = a.transpose(0, 1, 3, 2, 4)     # [n, h, p, i8, q]
        return a.reshape(5, 2, P, 4096)

    wkv5 = pack_kv(np.concatenate([Wk, Wv], axis=1))
    widkv5 = pack_kv(np.concatenate([Wid_k, Wid_v], axis=1))
    wkvh = np.ascontiguousarray(
        np.stack([(wkv5 if pr == 0 else widkv5)[n] for (pr, n) in KV_PLAN])
        .astype(h16))

    # woutp[p][i*H+m] = Wout[i*128+p, m]
    woutp = np.ascontiguousarray(
        Wout.reshape(NJ, P, H).transpose(1, 0, 2).reshape(P, NJ * H).astype(h16))
    identm = np.eye(P, dtype=h16)

    in_maps = []
    for core in range(NCORES):
        b, hf = divmod(core, 2)
        hsT = hidden_states[b, hf * SC:(hf + 1) * SC, :].T  # [H, SC]
        # hsTp[c][p][i*512+q] = hsT[i*128+p, c*512+q]
        hsTp = np.ascontiguousarray(
            hsT.reshape(NJ, P, NCH, 512).transpose(2, 1, 0, 3)
            .reshape(NCH, P, NJ * 512).astype(h16))
        xkvT = np.zeros((C, LP), h16)
        xkvT[:, :TE] = encoder_hidden_states[b].T
        xkvT[:, GAP1:] = id_embedding[b % 2].T
        # xkvp[p][i*128+l] = xkvT[i*128+p, l]
        xkvp = np.ascontiguousarray(
            xkvT.reshape(NI, P, LP).transpose(1, 0, 2).reshape(P, NI * LP))
        in_maps.append({
            "ident": identm, "hsTp": hsTp, "xkvp": xkvp, "wqp": wqp,
            "wkvh": wkvh, "woutp": woutp, "boutb": boutb,
        })
    return in_maps


def kernel(hidden_states, encoder_hidden_states, id_embedding,
           Wq, Wk, Wv, Wid_k, Wid_v, Wout, bout, _trace=False):
    _ensure_axon_hooks()
    from concourse.bass_utils import run_bass_kernel_spmd

    nc = build_nc()
    in_maps = prep_core_inputs(hidden_states, encoder_hidden_states, id_embedding,
                               Wq, Wk, Wv, Wid_k, Wid_v, Wout, bout)
    kwargs = {}
    if _trace:
        import concourse.bass_utils as bu
        bu.upload_artifacts = lambda tmpdir: f"local://{tmpdir}"
        kwargs["trace"] = True
    res = run_bass_kernel_spmd(nc, in_maps, core_ids=list(range(NCORES)), **kwargs)

    outp = np.empty((B, S, H), np.float32)
    for core in range(NCORES):
        b, hf = divmod(core, 2)
        outp[b, hf * SC:(hf + 1) * SC, :] = res.results[core]["out"]
    if _trace:
        kernel.last_exec_time_ns = res.exec_time_ns
        kernel.last_results = res
    return outp
